# revision 4
# baseline (speedup 1.0000x reference)
"""Bahdanau attention Trainium2 kernel.

reference (per batch row b):
    q_proj = query @ W1 + b1                       # [U]
    v_proj = values[b] @ W2 + b2                   # [L, U]
    score  = tanh(q_proj + v_proj) @ V + bv        # [L, 1]
    attn   = softmax(score, axis=0)                # [L, 1]
    ctx    = sum(attn * values[b], axis=0)         # [H]

Sharding: data-parallel over batch, 8 rows per NeuronCore (B=64 over 8 cores).
W1/W2/V replicated.  bv and the softmax are shift-invariant so bv is ignored;
b1/b2 are folded into the tanh bias.

Per-core pipeline (B_loc=8, n = B_loc*L = 16384):
  - values rows stream in once as f32 and are cast to bf16 during the DMA
    (SWDGE cast).  Natural [l,h] tiles stay resident for the whole batch row.
  - one DMA-transpose (xbar, 2-byte) per natural tile produces the [h, n]
    operand for the W2 projection.
  - PE: v_projT[u, n] accumulated over h in PSUM; ScalarE applies
    tanh(. + q_projT[u,b]+b1+b2) with the per-partition bias port, output is
    rounded to f32r.
  - score[1, n] = V.T @ tanh(...) as f32r matmuls (full PE rate at N=512).
  - softmax per batch row on partition 0; attention weights go straight to
    DRAM; wT columns come from PE transposes of 128-wide weight blocks.
  - context[1, h] = wT.T @ values_nat accumulated over the 16 resident
    natural tiles (bf16), so values is never re-read from HBM.
"""

import sys

sys.path.insert(0, "/opt/trn_rl_repo")

import numpy as np

B, L, H, U = 64, 2048, 1024, 1024
NCORES = 8
BL = B // NCORES          # batch rows per core
NLOC = BL * L             # 16384 values rows per core
CHUNK = 1024              # n columns handled per main-loop chunk
CPB = L // CHUNK          # chunks per batch row (2)
SUB = CHUNK // 128        # natural 128-row tiles per chunk (8)
HT = H // 128             # h tiles (8)
UT = U // 128             # u tiles (8)
LT = L // 128             # l tiles per batch row (16)

_CACHE = {}


def _build():
    import concourse.tile as tile
    from concourse import bacc, mybir

    f32 = mybir.dt.float32
    bf16 = mybir.dt.bfloat16
    f32r = mybir.dt.float32r
    AF = mybir.ActivationFunctionType

    nc = bacc.Bacc("TRN2", target_bir_lowering=False, debug=False)

    query = nc.dram_tensor("query", [BL, H], f32, kind="ExternalInput")
    values = nc.dram_tensor("values", [NLOC, H], f32, kind="ExternalInput")
    W1 = nc.dram_tensor("W1", [H, U], f32, kind="ExternalInput")
    b1 = nc.dram_tensor("b1", [U], f32, kind="ExternalInput")
    W2 = nc.dram_tensor("W2", [H, U], f32, kind="ExternalInput")
    b2 = nc.dram_tensor("b2", [U], f32, kind="ExternalInput")
    V = nc.dram_tensor("V", [U, 1], f32, kind="ExternalInput")
    out_ctx = nc.dram_tensor("out_ctx", [BL, H], f32, kind="ExternalOutput")
    out_attn = nc.dram_tensor("out_attn", [BL, L], f32, kind="ExternalOutput")

    with tile.TileContext(nc) as tc:
        with (
            tc.tile_pool(name="const", bufs=1) as const,
            tc.tile_pool(name="w1s", bufs=2) as w1_pool,
            tc.tile_pool(name="vna", bufs=2 * SUB * CPB) as vna_pool,
            tc.tile_pool(name="vt", bufs=2) as vt_pool,
            tc.tile_pool(name="s", bufs=3) as s_pool,
            tc.tile_pool(name="p0", bufs=2) as p0_pool,
            tc.tile_pool(name="ps_v", bufs=2, space="PSUM") as ps_v,
            tc.tile_pool(name="ps_score", bufs=2, space="PSUM") as ps_score,
            tc.tile_pool(name="ps_ctx", bufs=1, space="PSUM") as ps_ctx,
            tc.tile_pool(name="ps_wt", bufs=2, space="PSUM") as ps_wt,
        ):
            # ---- prologue: weights + q projection -------------------------
            w2bf = const.tile([128, HT, U], bf16)
            for t in range(HT):
                nc.gpsimd.dma_start(w2bf[:, t, :], W2[t * 128 : (t + 1) * 128, :])
            vsb = const.tile([128, UT], f32r)
            nc.gpsimd.dma_start(vsb[:], V[:, 0].rearrange("(t p) -> p t", p=128))
            b1t = const.tile([128, UT], f32)
            nc.sync.dma_start(b1t[:], b1[:].rearrange("(t p) -> p t", p=128))
            b2t = const.tile([128, UT], f32)
            nc.sync.dma_start(b2t[:], b2[:].rearrange("(t p) -> p t", p=128))
            qT = const.tile([128, HT, BL], f32r)
            for t in range(HT):
                nc.gpsimd.dma_start(
                    qT[:, t, :],
                    query[:, t * 128 : (t + 1) * 128].rearrange("b h -> h b"),
                )
            ident = const.tile([128, 128], f32)
            from concourse.masks import make_identity

            make_identity(nc, ident[:])

            # q_projT[u, b] + b1 + b2, laid out [128, u_tile, b]
            qb = const.tile([128, UT, BL], f32)
            for t in range(UT):
                qp = ps_wt.tile([128, BL], f32, tag="wt")
                for h in range(HT):
                    w1t = w1_pool.tile([128, 128], f32r, tag="w1")
                    nc.gpsimd.dma_start(
                        w1t[:], W1[h * 128 : (h + 1) * 128, t * 128 : (t + 1) * 128]
                    )
                    nc.tensor.matmul(
                        qp[:],
                        w1t[:],
                        qT[:, h, :],
                        start=(h == 0),
                        stop=(h == HT - 1),
                    )
                nc.vector.tensor_scalar(
                    qb[:, t, :],
                    qp[:],
                    b1t[:, t : t + 1],
                    b2t[:, t : t + 1],
                    mybir.AluOpType.add,
                    mybir.AluOpType.add,
                )

            # ---- main loop ------------------------------------------------
            for b in range(BL):
                nat = []  # natural bf16 tiles resident for this batch row
                score_b = p0_pool.tile([1, L], f32, tag="score")
                for cc in range(CPB):
                    c = b * CPB + cc
                    # load + cast natural tiles, then transpose via xbar
                    vt = vt_pool.tile([128, HT, CHUNK], bf16)
                    ctiles = []
                    for s in range(SUB):
                        row0 = c * CHUNK + s * 128
                        vn = vna_pool.tile([128, H], bf16, tag="vna")
                        nc.gpsimd.dma_start(vn[:], values[row0 : row0 + 128, :])
                        ctiles.append(vn)
                        nc.sync.dma_start_transpose(
                            vt[:, :, s * 128 : (s + 1) * 128], vn[:]
                        )
                    nat.extend(ctiles)

                    # v_projT + tanh + score
                    for j in range(2):
                        sc_ps = ps_score.tile([1, 512], f32, tag="score")
                        for u in range(UT):
                            pv = ps_v.tile([128, 512], f32, tag="pv")
                            for h in range(HT):
                                nc.tensor.matmul(
                                    pv[:],
                                    w2bf[:, h, u * 128 : (u + 1) * 128],
                                    vt[:, h, j * 512 : (j + 1) * 512],
                                    start=(h == 0),
                                    stop=(h == HT - 1),
                                )
                            st = s_pool.tile([128, 512], f32r, tag="s")
                            nc.scalar.activation(
                                st[:], pv[:], AF.Tanh, bias=qb[:, u, b : b + 1]
                            )
                            nc.tensor.matmul(
                                sc_ps[:],
                                vsb[:, u : u + 1],
                                st[:],
                                start=(u == 0),
                                stop=(u == UT - 1),
                            )
                        nc.vector.tensor_copy(
                            score_b[:, cc * CHUNK + j * 512 : cc * CHUNK + (j + 1) * 512],
                            sc_ps[:],
                        )

                # ---- softmax over L on partition 0 ------------------------
                mx = p0_pool.tile([1, 1], f32, tag="mx")
                nc.vector.reduce_max(mx[:], score_b[:], axis=mybir.AxisListType.X)
                nmx = p0_pool.tile([1, 1], f32, tag="nmx")
                nc.vector.tensor_scalar_mul(nmx[:], mx[:], -1.0)
                ex = p0_pool.tile([1, L], f32, tag="ex")
                nc.scalar.activation(ex[:], score_b[:], AF.Exp, bias=nmx[:])
                sm = p0_pool.tile([1, 1], f32, tag="sm")
                nc.vector.reduce_sum(sm[:], ex[:], axis=mybir.AxisListType.X)
                rs = p0_pool.tile([1, 1], f32, tag="rs")
                nc.vector.reciprocal(rs[:], sm[:])
                wrow = p0_pool.tile([1, L], f32, tag="wrow")
                nc.vector.tensor_scalar_mul(wrow[:], ex[:], rs[:])
                nc.sync.dma_start(out_attn[b : b + 1, :], wrow[:])

                # ---- wT columns + context --------------------------------
                wt = p0_pool.tile([128, LT], bf16, tag="wt")
                for t in range(LT):
                    wtp = ps_wt.tile([128, 1], f32, tag="wt")
                    nc.tensor.transpose(
                        wtp[:], wrow[:, t * 128 : (t + 1) * 128], ident[:1, :1]
                    )
                    nc.vector.tensor_copy(wt[:, t : t + 1], wtp[:])

                ctx_ps = ps_ctx.tile([1, H], f32, tag="ctx")
                for j in range(2):
                    for t in range(LT):
                        nc.tensor.matmul(
                            ctx_ps[:, j * 512 : (j + 1) * 512],
                            wt[:, t : t + 1],
                            nat[t][:, j * 512 : (j + 1) * 512],
                            start=(t == 0),
                            stop=(t == LT - 1),
                        )
                ctx_sb = p0_pool.tile([1, H], f32, tag="ctx_sb")
                nc.vector.tensor_copy(ctx_sb[:], ctx_ps[:])
                nc.sync.dma_start(out_ctx[b : b + 1, :], ctx_sb[:])

    nc.compile()
    return nc


def _get_nc():
    if "nc" not in _CACHE:
        _CACHE["nc"] = _build()
    return _CACHE["nc"]


def make_in_maps(query, values, W1, b1, W2, b2, V):
    query = np.ascontiguousarray(np.asarray(query, dtype=np.float32))
    values = np.ascontiguousarray(np.asarray(values, dtype=np.float32))
    reps = {
        "W1": np.ascontiguousarray(np.asarray(W1, dtype=np.float32)),
        "b1": np.ascontiguousarray(np.asarray(b1, dtype=np.float32)),
        "W2": np.ascontiguousarray(np.asarray(W2, dtype=np.float32)),
        "b2": np.ascontiguousarray(np.asarray(b2, dtype=np.float32)),
        "V": np.ascontiguousarray(np.asarray(V, dtype=np.float32)),
    }
    in_maps = []
    for i in range(NCORES):
        sl = slice(i * BL, (i + 1) * BL)
        in_maps.append(
            {
                "query": query[sl],
                "values": values[sl].reshape(NLOC, H),
                **reps,
            }
        )
    return in_maps


def assemble(results):
    ctx = np.concatenate([r["out_ctx"] for r in results], axis=0)
    attn = np.concatenate([r["out_attn"] for r in results], axis=0)
    return ctx, attn.reshape(B, L, 1)


def kernel(query, values, W1, b1, W2, b2, V, bv):
    from concourse.bass_utils import run_bass_kernel_spmd

    # bv shifts every score of a batch row equally; softmax is shift-invariant
    # so neither output depends on it.
    nc = _get_nc()
    in_maps = make_in_maps(query, values, W1, b1, W2, b2, V)
    res = run_bass_kernel_spmd(nc, in_maps, list(range(NCORES)))
    return assemble(res.results)
